# revision 37
# baseline (speedup 1.0000x reference)
"""Sliding-window GQA attention (soft-capped) on 8 TRN2 NeuronCores.

Problem: B=2, S=2048, H=32 q-heads, H_KV=8 kv-heads, D=128, causal sliding
window 1024, logits soft-cap 30*tanh(s/30), scale 1/sqrt(D).

Sharding: head-parallel. Core c gets kv head c and q heads [4c, 4c+4) —
fully independent per core, no collectives.

Per-core algorithm (all on one NeuronCore, Tile-scheduled):
  - Q^T/K^T layouts ([d, s]) built on-chip via PE transposes, cast to bf16.
  - Scores computed TRANSPOSED: for each k-tile kt, one strip
    S^T[k=128, q window <=1152] = K_tile^T.T @ Q^T — avoids transposing
    probabilities for the PV matmul.
  - Soft-cap+softmax without max-subtraction (logits bounded by +-30):
    t = tanh(s * scale/30) on ScalarE (PSUM->SBUF), E = exp(30 t) on
    ScalarE (-> bf16). Causal/window masks: multiply 2 boundary 128-col
    blocks by 0/1 masks on VectorE.
  - For each 512-wide q-chunk: num^T[d, q] = sum_kt V_kt.T.T @ E_strip
    accumulated in PSUM (per-element has_written handles the staggered
    strip windows); den[q] replicated across partitions via an all-ones
    stationary matmul. out = (num/den) transposed back via PE.
"""

import numpy as np

import concourse.bass as bass
import concourse.mybir as mybir
import concourse.tile as tile
from concourse import bacc
from concourse import bass_utils as _bu
from concourse.bass_utils import run_bass_kernel_spmd
from concourse.masks import make_identity



AF = mybir.ActivationFunctionType
F32 = mybir.dt.float32
BF16 = mybir.dt.bfloat16

P = 128  # head dim == partition count == seq tile
B = 2
S = 2048
QH = 4  # q heads per core
NT = S // P  # 16 seq tiles
W = 1024  # sliding window
MAXW = W + P  # max strip width (9 tiles)
CHUNK = 512
NCH = S // CHUNK  # q-chunks per (b, head)
SCALE = 1.0 / np.sqrt(128.0)
CAP = 30.0
N_CORES = 8


def _strip_width(kt: int) -> int:
    return min(MAXW, S - kt * P)


def build_core_graph():
    nc = bacc.Bacc("TRN2", target_bir_lowering=False, debug=False, num_devices=N_CORES)
    q_ext = nc.declare_dram_parameter("query", [B, S, QH * P], F32, isOutput=False)
    k_ext = nc.declare_dram_parameter("key", [B, S, P], F32, isOutput=False)
    v_ext = nc.declare_dram_parameter("value", [B, S, P], F32, isOutput=False)
    # out stays in the on-chip [d, q] orientation; the host permutes to
    # [B, S, QH*P] while unsharding.
    out_ext = nc.declare_dram_parameter("out", [B, QH, P, S], F32, isOutput=True)

    with tile.TileContext(nc) as tc:
        with (
            tc.tile_pool(name="const", bufs=1) as constp,
            tc.tile_pool(name="persist", bufs=1) as pp,
        ):
            ident = constp.tile([P, P], F32, name="ident", tag="ident")
            make_identity(nc, ident[:])
            ident_bf = constp.tile([P, P], BF16, name="identbf", tag="identbf")
            make_identity(nc, ident_bf[:])
            # den only needs one row (gpsimd broadcasts it); a 1-col
            # stationary makes its LDWEIGHTS nearly free.
            ones_bf = constp.tile([P, 1], BF16, name="ones", tag="ones")
            nc.vector.memset(ones_bf[:], 1.0)
            # Strip coords: row r = k offset, col c = q offset (q-k = c-r).
            # m1 (first 128 cols): keep c >= r (causal).
            m1 = constp.tile([P, P], BF16, name="m1", tag="m1")
            nc.gpsimd.memset(m1[:], 1.0)
            nc.gpsimd.affine_select(
                out=m1[:],
                in_=m1[:],
                compare_op=mybir.AluOpType.is_ge,
                fill=0.0,
                base=0,
                pattern=[[1, P]],
                channel_multiplier=-1,
            )
            # m2 (cols [1024,1152)): keep c' < r (window cutoff at c-r=1024).
            m2 = constp.tile([P, P], BF16, name="m2", tag="m2")
            nc.gpsimd.memset(m2[:], 1.0)
            nc.gpsimd.affine_select(
                out=m2[:],
                in_=m2[:],
                compare_op=mybir.AluOpType.is_gt,
                fill=0.0,
                base=0,
                pattern=[[-1, P]],
                channel_multiplier=1,
            )

            # dummy transcendentals up front so the ~2.7us ACT table load
            # lands in the startup shadow, not before the first real tanh
            warm = constp.tile([P, 1], F32, name="warm", tag="warm")
            nc.scalar.activation(warm[:], ones_bf[:, 0:1], AF.Tanh)
            nc.scalar.activation(warm[:], warm[:], AF.Exp)

            # Persistent bf16 layouts. qT_all[b] holds the 4 heads
            # concatenated: head h occupies cols [h*S, (h+1)*S).
            qT_all = [
                pp.tile([P, QH * S], BF16, name=f"qT{b}", tag=f"qT{b}") for b in range(B)
            ]
            qT = [
                [qT_all[b][:, h * S : (h + 1) * S] for h in range(QH)] for b in range(B)
            ]
            kT = [pp.tile([P, S], BF16, name=f"kT{b}", tag=f"kT{b}") for b in range(B)]
            vB = [pp.tile([P, S], BF16, name=f"vB{b}", tag=f"vB{b}") for b in range(B)]

            # ---- main loop (loads/transposes folded in on demand so the
            # first strip starts as soon as ~9 Q tiles have landed) ----
            with (
                tc.tile_pool(name="load", bufs=4) as loadp,
                tc.tile_pool(name="spsum", bufs=2, space="PSUM") as sp,
                tc.tile_pool(name="apsum", bufs=2, space="PSUM") as auxp,
                tc.tile_pool(name="tbuf", bufs=2) as tbp,
                tc.tile_pool(name="ebuf", bufs=13) as ebp,
                tc.tile_pool(name="misc", bufs=2) as mp,
            ):
                loaded = set()
                loaded_k = set()

                def ensure_k(b, t):
                    if (b, t) in loaded_k or t >= NT:
                        return
                    loaded_k.add((b, t))
                    rows = slice(t * P, (t + 1) * P)
                    ktile = loadp.tile([P, P], F32, name="kload", tag="kload")
                    nc.sync.dma_start(out=ktile[:], in_=k_ext[b, rows, :])
                    kcast = loadp.tile([P, P], BF16, name="kcast", tag="kcast")
                    nc.vector.tensor_copy(kcast[:], ktile[:])
                    kps = auxp.tile([P, P], BF16, name="kps", tag="aux")
                    nc.tensor.transpose(kps[:], kcast[:], ident_bf[:])
                    nc.vector.tensor_copy(kT[b][:, t * P : (t + 1) * P], kps[:])

                def fast_start(b, nt9=9):
                    # startup: head-0 Q and K for tiles [0, nt9) in ONE
                    # strided DMA each, then xbar-transpose tile-by-tile.
                    q_re = q_ext[b].rearrange("(t p) d -> p t d", p=P)[:, 0:nt9, 0:P]
                    k_re = k_ext[b].rearrange("(t p) d -> p t d", p=P)[:, 0:nt9, :]
                    for name, src, dst_all in (
                        ("qf", q_re, qT_all[b]),
                        ("kf", k_re, kT[b]),
                    ):
                        raw = loadp.tile([P, nt9 * P], F32, name=f"{name}raw", tag="qload")
                        nc.sync.dma_start(
                            out=raw.rearrange("p (t d) -> p t d", d=P), in_=src
                        )
                        cast = loadp.tile([P, nt9 * P], BF16, name=f"{name}c", tag="qcast")
                        nc.vector.tensor_copy(cast[:], raw[:])
                        for g in range(0, nt9, 4):
                            gn = min(4, nt9 - g)
                            ps = auxp.tile([P, 4 * P], BF16, name=f"{name}ps", tag="aux")
                            for j in range(gn):
                                t = g + j
                                nc.tensor.transpose(
                                    ps[:, j * P : (j + 1) * P],
                                    cast[:, t * P : (t + 1) * P],
                                    ident_bf[:],
                                )
                            nc.vector.tensor_copy(
                                dst_all[:, g * P : (g + gn) * P], ps[:, : gn * P]
                            )
                    for t in range(nt9):
                        loaded_k.add((b, t))

                def ensure_tile(b, t, h0=0):
                    if (b, t) in loaded or t >= NT:
                        return
                    loaded.add((b, t))
                    nh = QH - h0
                    rows = slice(t * P, (t + 1) * P)
                    qtile = loadp.tile([P, nh * P], F32, name="qload", tag="qload")
                    nc.sync.dma_start(out=qtile[:], in_=q_ext[b, rows, h0 * P :])
                    qcast = loadp.tile([P, nh * P], BF16, name="qcast", tag="qcast")
                    nc.vector.tensor_copy(qcast[:], qtile[:])
                    qps = auxp.tile([P, nh * P], BF16, name="qps", tag="aux")
                    for i in range(nh):
                        nc.tensor.transpose(
                            qps[:, i * P : (i + 1) * P],
                            qcast[:, i * P : (i + 1) * P],
                            ident_bf[:],
                        )
                    # one strided copy scatters the head blocks
                    dst = qT_all[b].rearrange("p (h s) -> p h s", h=QH)[
                        :, h0:, t * P : (t + 1) * P
                    ]
                    nc.vector.tensor_copy(
                        dst, qps.rearrange("p (h s) -> p h s", h=nh)
                    )
                    ensure_k(b, t)
                    vtile = loadp.tile([P, P], F32, name="vload", tag="vload")
                    nc.sync.dma_start(out=vtile[:], in_=v_ext[b, rows, :])
                    nc.vector.tensor_copy(vB[b][:, t * P : (t + 1) * P], vtile[:])

                estrips = {}  # (b,h,kt) -> (epair_tile, col offset)
                tpairs = {}  # (b,h,pair) -> (tpair_tile, epair_tile)

                def emit_strip(b, h, kt):
                    w = _strip_width(kt)
                    q0s = kt * P  # strip q origin
                    strip = sp.tile([P, MAXW], F32, name="strip", tag="strip")
                    for c0 in range(0, w, CHUNK):
                        c1 = min(c0 + CHUNK, w)
                        nc.tensor.matmul(
                            strip[:, c0:c1],
                            lhsT=kT[b][:, q0s : q0s + P],
                            rhs=qT[b][h][:, q0s + c0 : q0s + c1],
                            start=True,
                            stop=True,
                        )
                    if kt % 2 == 0:
                        tpair = tbp.tile([P, 2 * MAXW], F32, name="t", tag="t")
                        epair = ebp.tile([P, 2 * MAXW], BF16, name="e", tag="e")
                        tpairs[(b, h, kt // 2)] = (tpair, epair)
                        off = 0
                    else:
                        tpair, epair = tpairs[(b, h, kt // 2)]
                        off = _strip_width(kt - 1)
                    estrips[(b, h, kt)] = (epair, off)
                    nc.scalar.activation(
                        tpair[:, off : off + w], strip[:, :w], AF.Tanh, scale=SCALE / CAP
                    )
                    if kt % 2 == 1:
                        # one merged exp for the strip pair, then masks
                        wtot = off + w
                        nc.scalar.activation(
                            epair[:, :wtot], tpair[:, :wtot], AF.Exp, scale=CAP
                        )
                        for k3 in (kt - 1, kt):
                            _, o3 = estrips[(b, h, k3)]
                            w3 = _strip_width(k3)
                            nc.vector.tensor_mul(
                                epair[:, o3 : o3 + P], epair[:, o3 : o3 + P], m1[:]
                            )
                            if w3 > W:
                                nc.vector.tensor_mul(
                                    epair[:, o3 + W : o3 + W + P],
                                    epair[:, o3 + W : o3 + W + P],
                                    m2[:],
                                )

                def _chunk_mms(b, h, c, dst, lhs_of, half):
                    q0 = c * CHUNK
                    kts = list(range(max(0, 4 * c - 8), 4 * c + 4))
                    mid = (len(kts) + 1) // 2
                    sel = kts[:mid] if half == 0 else kts[mid:]
                    for k2 in sel:
                        s0 = max(q0, k2 * P)
                        s1 = min(q0 + CHUNK, k2 * P + _strip_width(k2))
                        col0 = s0 - k2 * P
                        n = s1 - s0
                        d0 = s0 - q0
                        lhs = lhs_of(k2)
                        etile, eoff = estrips[(b, h, k2)]
                        nc.tensor.matmul(
                            dst[: lhs.shape[1], d0 : d0 + n],
                            lhsT=lhs,
                            rhs=etile[:, eoff + col0 : eoff + col0 + n],
                            start=(k2 == kts[0]),
                            stop=(k2 == kts[-1]),
                            skip_group_check=True,
                        )

                def chunk_tail(st):
                    b, h, c = st["key"]
                    q0 = c * CHUNK
                    # normalize in the [d, q] orientation: reciprocal of the
                    # single den row, gpsimd-broadcast across partitions,
                    # one TT multiply draining num PSUM -> SBUF, one DMA out.
                    recip_row = mp.tile([1, CHUNK], F32, name="recip_row", tag="recip_row")
                    nc.vector.reciprocal_approx_fast(recip_row[:], st["den"][:])
                    recip_rep = mp.tile([P, CHUNK], F32, name="recip_rep", tag="recip_rep")
                    nc.gpsimd.partition_broadcast(recip_rep[:], recip_row[:])
                    ostage = mp.tile([P, CHUNK], F32, name="ostage", tag="ostage")
                    nc.vector.tensor_mul(ostage[:], st["num"][:], recip_rep[:])
                    nc.sync.dma_start(
                        out=out_ext[b, h, :, q0 : q0 + CHUNK],
                        in_=ostage[:],
                    )

                pending = []

                def advance_pending():
                    if not pending:
                        return
                    st = pending[0]
                    b, h, c = st["key"]
                    stage = st["stage"]
                    if stage == 0:
                        st["num"] = auxp.tile([P, CHUNK], F32, name="num", tag="aux")
                        _chunk_mms(b, h, c, st["num"],
                                   lambda k2: vB[b][:, k2 * P : (k2 + 1) * P], 0)
                    elif stage == 1:
                        _chunk_mms(b, h, c, st["num"],
                                   lambda k2: vB[b][:, k2 * P : (k2 + 1) * P], 1)
                    elif stage == 2:
                        st["den"] = auxp.tile([1, CHUNK], F32, name="den", tag="aux")
                        _chunk_mms(b, h, c, st["den"], lambda k2: ones_bf[:], 0)
                    else:
                        _chunk_mms(b, h, c, st["den"], lambda k2: ones_bf[:], 1)
                        chunk_tail(st)
                        pending.pop(0)
                        return
                    st["stage"] = stage + 1

                for b in range(B):
                    for h in range(QH):
                        for kt in range(NT):
                            if b == 0 and h == 0:
                                if kt == 0:
                                    fast_start(0)
                                else:
                                    ensure_tile(0, kt - 1, h0=1)
                                    ensure_tile(0, kt + 8)
                            elif h == 0 and kt > 0:
                                ensure_tile(b, kt + 8)
                            emit_strip(b, h, kt)
                            # chunk work trickles in between strips so PE
                            # never runs a long chunk block right before a
                            # strip ACT depends on
                            advance_pending()
                            if kt % 4 == 3:
                                pending.append({"key": (b, h, kt // 4), "stage": 0})
                            if h == QH - 1 and b + 1 < B:
                                ensure_tile(b + 1, 2 * kt)
                                ensure_tile(b + 1, 2 * kt + 1)
                while pending:
                    advance_pending()
    nc.compile()
    return nc


_NC_CACHE = [None]


def _get_nc():
    if _NC_CACHE[0] is None:
        _NC_CACHE[0] = build_core_graph()
    return _NC_CACHE[0]


def _shard(query, key, value):
    in_maps = []
    for c in range(N_CORES):
        in_maps.append(
            {
                "query": np.ascontiguousarray(
                    query[:, :, c * QH * P : (c + 1) * QH * P], dtype=np.float32
                ),
                "key": np.ascontiguousarray(
                    key[:, :, c * P : (c + 1) * P], dtype=np.float32
                ),
                "value": np.ascontiguousarray(
                    value[:, :, c * P : (c + 1) * P], dtype=np.float32
                ),
            }
        )
    return in_maps


def _run(query, key, value, trace=False):
    nc = _get_nc()
    in_maps = _shard(query, key, value)
    res = run_bass_kernel_spmd(nc, in_maps, core_ids=list(range(N_CORES)), trace=trace)
    out = np.empty((B, S, N_CORES * QH * P), dtype=np.float32)
    for c in range(N_CORES):
        # device output is [B, QH, P(d), S]; permute to [B, S, QH*P]
        o = res.results[c]["out"].transpose(0, 3, 1, 2).reshape(B, S, QH * P)
        out[:, :, c * QH * P : (c + 1) * QH * P] = o
    return out, res


def kernel(query, key, value):
    out, _ = _run(query, key, value, trace=False)
    return out


# revision 38
# speedup vs baseline: 1.0355x; 1.0355x over previous
"""Sliding-window GQA attention (soft-capped) on 8 TRN2 NeuronCores.

Problem: B=2, S=2048, H=32 q-heads, H_KV=8 kv-heads, D=128, causal sliding
window 1024, logits soft-cap 30*tanh(s/30), scale 1/sqrt(D).

Sharding: head-parallel. Core c gets kv head c and q heads [4c, 4c+4) —
fully independent per core, no collectives.

Per-core algorithm (all on one NeuronCore, Tile-scheduled):
  - Q^T/K^T layouts ([d, s]) built on-chip via PE transposes, cast to bf16.
  - Scores computed TRANSPOSED: for each k-tile kt, one strip
    S^T[k=128, q window <=1152] = K_tile^T.T @ Q^T — avoids transposing
    probabilities for the PV matmul.
  - Soft-cap+softmax without max-subtraction (logits bounded by +-30):
    t = tanh(s * scale/30) on ScalarE (PSUM->SBUF), E = exp(30 t) on
    ScalarE (-> bf16). Causal/window masks: multiply 2 boundary 128-col
    blocks by 0/1 masks on VectorE.
  - For each 512-wide q-chunk: num^T[d, q] = sum_kt V_kt.T.T @ E_strip
    accumulated in PSUM (per-element has_written handles the staggered
    strip windows); den[q] replicated across partitions via an all-ones
    stationary matmul. out = (num/den) transposed back via PE.
"""

import numpy as np

import concourse.bass as bass
import concourse.mybir as mybir
import concourse.tile as tile
from concourse import bacc
from concourse import bass_utils as _bu
from concourse.bass_utils import run_bass_kernel_spmd
from concourse.masks import make_identity



AF = mybir.ActivationFunctionType
F32 = mybir.dt.float32
BF16 = mybir.dt.bfloat16

P = 128  # head dim == partition count == seq tile
B = 2
S = 2048
QH = 4  # q heads per core
NT = S // P  # 16 seq tiles
W = 1024  # sliding window
MAXW = W + P  # max strip width (9 tiles)
CHUNK = 512
NCH = S // CHUNK  # q-chunks per (b, head)
SCALE = 1.0 / np.sqrt(128.0)
CAP = 30.0
N_CORES = 8


def _strip_width(kt: int) -> int:
    return min(MAXW, S - kt * P)


def build_core_graph():
    nc = bacc.Bacc("TRN2", target_bir_lowering=False, debug=False, num_devices=N_CORES)
    q_ext = nc.declare_dram_parameter("query", [B, S, QH * P], F32, isOutput=False)
    k_ext = nc.declare_dram_parameter("key", [B, S, P], F32, isOutput=False)
    v_ext = nc.declare_dram_parameter("value", [B, S, P], F32, isOutput=False)
    # out stays in the on-chip [d, q] orientation; the host permutes to
    # [B, S, QH*P] while unsharding.
    out_ext = nc.declare_dram_parameter("out", [B, QH, P, S], F32, isOutput=True)

    with tile.TileContext(nc) as tc:
        with (
            tc.tile_pool(name="const", bufs=1) as constp,
            tc.tile_pool(name="persist", bufs=1) as pp,
        ):
            ident = constp.tile([P, P], F32, name="ident", tag="ident")
            make_identity(nc, ident[:])
            ident_bf = constp.tile([P, P], BF16, name="identbf", tag="identbf")
            make_identity(nc, ident_bf[:])
            # den only needs one row (gpsimd broadcasts it); a 1-col
            # stationary makes its LDWEIGHTS nearly free.
            ones_bf = constp.tile([P, 1], BF16, name="ones", tag="ones")
            nc.vector.memset(ones_bf[:], 1.0)
            # Strip coords: row r = k offset, col c = q offset (q-k = c-r).
            # m1 (first 128 cols): keep c >= r (causal).
            m1 = constp.tile([P, P], BF16, name="m1", tag="m1")
            nc.gpsimd.memset(m1[:], 1.0)
            nc.gpsimd.affine_select(
                out=m1[:],
                in_=m1[:],
                compare_op=mybir.AluOpType.is_ge,
                fill=0.0,
                base=0,
                pattern=[[1, P]],
                channel_multiplier=-1,
            )
            # m2 (cols [1024,1152)): keep c' < r (window cutoff at c-r=1024).
            m2 = constp.tile([P, P], BF16, name="m2", tag="m2")
            nc.gpsimd.memset(m2[:], 1.0)
            nc.gpsimd.affine_select(
                out=m2[:],
                in_=m2[:],
                compare_op=mybir.AluOpType.is_gt,
                fill=0.0,
                base=0,
                pattern=[[-1, P]],
                channel_multiplier=1,
            )

            # dummy transcendentals up front so the ~2.7us ACT table load
            # lands in the startup shadow, not before the first real tanh
            warm = constp.tile([P, 1], F32, name="warm", tag="warm")
            nc.scalar.activation(warm[:], ones_bf[:, 0:1], AF.Tanh)
            nc.scalar.activation(warm[:], warm[:], AF.Exp)

            # Persistent bf16 layouts. qT_all[b] holds the 4 heads
            # concatenated: head h occupies cols [h*S, (h+1)*S).
            qT_all = [
                pp.tile([P, QH * S], BF16, name=f"qT{b}", tag=f"qT{b}") for b in range(B)
            ]
            qT = [
                [qT_all[b][:, h * S : (h + 1) * S] for h in range(QH)] for b in range(B)
            ]
            kT = [pp.tile([P, S], BF16, name=f"kT{b}", tag=f"kT{b}") for b in range(B)]
            vB = [pp.tile([P, S], BF16, name=f"vB{b}", tag=f"vB{b}") for b in range(B)]

            # ---- main loop (loads/transposes folded in on demand so the
            # first strip starts as soon as ~9 Q tiles have landed) ----
            with (
                tc.tile_pool(name="load", bufs=6) as loadp,
                tc.tile_pool(name="spsum", bufs=2, space="PSUM") as sp,
                tc.tile_pool(name="apsum", bufs=2, space="PSUM") as auxp,
                tc.tile_pool(name="tbuf", bufs=2) as tbp,
                tc.tile_pool(name="ebuf", bufs=13) as ebp,
                tc.tile_pool(name="misc", bufs=2) as mp,
            ):
                loaded = set()
                loaded_k = set()

                def ensure_k(b, t):
                    if (b, t) in loaded_k or t >= NT:
                        return
                    loaded_k.add((b, t))
                    rows = slice(t * P, (t + 1) * P)
                    ktile = loadp.tile([P, P], F32, name="kload", tag="kload")
                    nc.sync.dma_start(out=ktile[:], in_=k_ext[b, rows, :])
                    kcast = loadp.tile([P, P], BF16, name="kcast", tag="kcast")
                    nc.vector.tensor_copy(kcast[:], ktile[:])
                    kps = auxp.tile([P, P], BF16, name="kps", tag="aux")
                    nc.tensor.transpose(kps[:], kcast[:], ident_bf[:])
                    nc.vector.tensor_copy(kT[b][:, t * P : (t + 1) * P], kps[:])

                def fast_start(b, nt9=9):
                    # startup: head-0 Q and K for tiles [0, nt9) in ONE
                    # strided DMA each, then xbar-transpose tile-by-tile.
                    q_re = q_ext[b].rearrange("(t p) d -> p t d", p=P)[:, 0:nt9, 0:P]
                    k_re = k_ext[b].rearrange("(t p) d -> p t d", p=P)[:, 0:nt9, :]
                    for name, src, dst_all in (
                        ("qf", q_re, qT_all[b]),
                        ("kf", k_re, kT[b]),
                    ):
                        raw = loadp.tile([P, nt9 * P], F32, name=f"{name}raw", tag="qload")
                        nc.sync.dma_start(
                            out=raw.rearrange("p (t d) -> p t d", d=P), in_=src
                        )
                        cast = loadp.tile([P, nt9 * P], BF16, name=f"{name}c", tag="qcast")
                        nc.vector.tensor_copy(cast[:], raw[:])
                        for g in range(0, nt9, 4):
                            gn = min(4, nt9 - g)
                            ps = auxp.tile([P, 4 * P], BF16, name=f"{name}ps", tag="aux")
                            for j in range(gn):
                                t = g + j
                                nc.tensor.transpose(
                                    ps[:, j * P : (j + 1) * P],
                                    cast[:, t * P : (t + 1) * P],
                                    ident_bf[:],
                                )
                            nc.vector.tensor_copy(
                                dst_all[:, g * P : (g + gn) * P], ps[:, : gn * P]
                            )
                    for t in range(nt9):
                        loaded_k.add((b, t))

                def ensure_tile(b, t, h0=0):
                    if (b, t) in loaded or t >= NT:
                        return
                    loaded.add((b, t))
                    nh = QH - h0
                    rows = slice(t * P, (t + 1) * P)
                    qtile = loadp.tile([P, nh * P], F32, name="qload", tag="qload")
                    nc.sync.dma_start(out=qtile[:], in_=q_ext[b, rows, h0 * P :])
                    qcast = loadp.tile([P, nh * P], BF16, name="qcast", tag="qcast")
                    nc.vector.tensor_copy(qcast[:], qtile[:])
                    qps = auxp.tile([P, nh * P], BF16, name="qps", tag="aux")
                    for i in range(nh):
                        nc.tensor.transpose(
                            qps[:, i * P : (i + 1) * P],
                            qcast[:, i * P : (i + 1) * P],
                            ident_bf[:],
                        )
                    # one strided copy scatters the head blocks
                    dst = qT_all[b].rearrange("p (h s) -> p h s", h=QH)[
                        :, h0:, t * P : (t + 1) * P
                    ]
                    nc.vector.tensor_copy(
                        dst, qps.rearrange("p (h s) -> p h s", h=nh)
                    )
                    ensure_k(b, t)
                    vtile = loadp.tile([P, P], F32, name="vload", tag="vload")
                    nc.sync.dma_start(out=vtile[:], in_=v_ext[b, rows, :])
                    nc.vector.tensor_copy(vB[b][:, t * P : (t + 1) * P], vtile[:])

                estrips = {}  # (b,h,kt) -> (epair_tile, col offset)
                tpairs = {}  # (b,h,pair) -> (tpair_tile, epair_tile)

                def emit_strip(b, h, kt):
                    w = _strip_width(kt)
                    q0s = kt * P  # strip q origin
                    strip = sp.tile([P, MAXW], F32, name="strip", tag="strip")
                    for c0 in range(0, w, CHUNK):
                        c1 = min(c0 + CHUNK, w)
                        nc.tensor.matmul(
                            strip[:, c0:c1],
                            lhsT=kT[b][:, q0s : q0s + P],
                            rhs=qT[b][h][:, q0s + c0 : q0s + c1],
                            start=True,
                            stop=True,
                        )
                    if kt % 2 == 0:
                        tpair = tbp.tile([P, 2 * MAXW], F32, name="t", tag="t")
                        epair = ebp.tile([P, 2 * MAXW], BF16, name="e", tag="e")
                        tpairs[(b, h, kt // 2)] = (tpair, epair)
                        off = 0
                    else:
                        tpair, epair = tpairs[(b, h, kt // 2)]
                        off = _strip_width(kt - 1)
                    estrips[(b, h, kt)] = (epair, off)
                    nc.scalar.activation(
                        tpair[:, off : off + w], strip[:, :w], AF.Tanh, scale=SCALE / CAP
                    )
                    if kt % 2 == 1:
                        # one merged exp for the strip pair, then masks
                        wtot = off + w
                        nc.scalar.activation(
                            epair[:, :wtot], tpair[:, :wtot], AF.Exp, scale=CAP
                        )
                        for k3 in (kt - 1, kt):
                            _, o3 = estrips[(b, h, k3)]
                            w3 = _strip_width(k3)
                            nc.vector.tensor_mul(
                                epair[:, o3 : o3 + P], epair[:, o3 : o3 + P], m1[:]
                            )
                            if w3 > W:
                                nc.vector.tensor_mul(
                                    epair[:, o3 + W : o3 + W + P],
                                    epair[:, o3 + W : o3 + W + P],
                                    m2[:],
                                )

                def _chunk_mms(b, h, c, dst, lhs_of, half):
                    q0 = c * CHUNK
                    kts = list(range(max(0, 4 * c - 8), 4 * c + 4))
                    mid = (len(kts) + 1) // 2
                    sel = kts[:mid] if half == 0 else kts[mid:]
                    for k2 in sel:
                        s0 = max(q0, k2 * P)
                        s1 = min(q0 + CHUNK, k2 * P + _strip_width(k2))
                        col0 = s0 - k2 * P
                        n = s1 - s0
                        d0 = s0 - q0
                        lhs = lhs_of(k2)
                        etile, eoff = estrips[(b, h, k2)]
                        nc.tensor.matmul(
                            dst[: lhs.shape[1], d0 : d0 + n],
                            lhsT=lhs,
                            rhs=etile[:, eoff + col0 : eoff + col0 + n],
                            start=(k2 == kts[0]),
                            stop=(k2 == kts[-1]),
                            skip_group_check=True,
                        )

                def chunk_tail(st):
                    b, h, c = st["key"]
                    q0 = c * CHUNK
                    # normalize in the [d, q] orientation: reciprocal of the
                    # single den row, gpsimd-broadcast across partitions,
                    # one TT multiply draining num PSUM -> SBUF, one DMA out.
                    recip_row = mp.tile([1, CHUNK], F32, name="recip_row", tag="recip_row")
                    nc.vector.reciprocal_approx_fast(recip_row[:], st["den"][:])
                    recip_rep = mp.tile([P, CHUNK], F32, name="recip_rep", tag="recip_rep")
                    nc.gpsimd.partition_broadcast(recip_rep[:], recip_row[:])
                    ostage = mp.tile([P, CHUNK], F32, name="ostage", tag="ostage")
                    nc.vector.tensor_mul(ostage[:], st["num"][:], recip_rep[:])
                    nc.sync.dma_start(
                        out=out_ext[b, h, :, q0 : q0 + CHUNK],
                        in_=ostage[:],
                    )

                pending = []

                def advance_pending():
                    if not pending:
                        return
                    st = pending[0]
                    b, h, c = st["key"]
                    stage = st["stage"]
                    if stage == 0:
                        st["num"] = auxp.tile([P, CHUNK], F32, name="num", tag="aux")
                        _chunk_mms(b, h, c, st["num"],
                                   lambda k2: vB[b][:, k2 * P : (k2 + 1) * P], 0)
                    elif stage == 1:
                        _chunk_mms(b, h, c, st["num"],
                                   lambda k2: vB[b][:, k2 * P : (k2 + 1) * P], 1)
                    elif stage == 2:
                        st["den"] = auxp.tile([1, CHUNK], F32, name="den", tag="aux")
                        _chunk_mms(b, h, c, st["den"], lambda k2: ones_bf[:], 0)
                    else:
                        _chunk_mms(b, h, c, st["den"], lambda k2: ones_bf[:], 1)
                        chunk_tail(st)
                        pending.pop(0)
                        return
                    st["stage"] = stage + 1

                for b in range(B):
                    for h in range(QH):
                        for kt in range(NT):
                            if b == 0 and h == 0:
                                if kt == 0:
                                    fast_start(0)
                                else:
                                    ensure_tile(0, kt - 1, h0=1)
                                    ensure_tile(0, kt + 8)
                            elif h == 0 and kt > 0:
                                ensure_tile(b, kt + 8)
                            emit_strip(b, h, kt)
                            # chunk work trickles in between strips so PE
                            # never runs a long chunk block right before a
                            # strip ACT depends on
                            advance_pending()
                            if kt % 4 == 3:
                                pending.append({"key": (b, h, kt // 4), "stage": 0})
                            if h == QH - 2 and b + 1 < B:
                                ensure_tile(b + 1, 2 * kt)
                                ensure_tile(b + 1, 2 * kt + 1)
                while pending:
                    advance_pending()
    nc.compile()
    return nc


_NC_CACHE = [None]


def _get_nc():
    if _NC_CACHE[0] is None:
        _NC_CACHE[0] = build_core_graph()
    return _NC_CACHE[0]


def _shard(query, key, value):
    in_maps = []
    for c in range(N_CORES):
        in_maps.append(
            {
                "query": np.ascontiguousarray(
                    query[:, :, c * QH * P : (c + 1) * QH * P], dtype=np.float32
                ),
                "key": np.ascontiguousarray(
                    key[:, :, c * P : (c + 1) * P], dtype=np.float32
                ),
                "value": np.ascontiguousarray(
                    value[:, :, c * P : (c + 1) * P], dtype=np.float32
                ),
            }
        )
    return in_maps


def _run(query, key, value, trace=False):
    nc = _get_nc()
    in_maps = _shard(query, key, value)
    res = run_bass_kernel_spmd(nc, in_maps, core_ids=list(range(N_CORES)), trace=trace)
    out = np.empty((B, S, N_CORES * QH * P), dtype=np.float32)
    for c in range(N_CORES):
        # device output is [B, QH, P(d), S]; permute to [B, S, QH*P]
        o = res.results[c]["out"].transpose(0, 3, 1, 2).reshape(B, S, QH * P)
        out[:, :, c * QH * P : (c + 1) * QH * P] = o
    return out, res


def kernel(query, key, value):
    out, _ = _run(query, key, value, trace=False)
    return out


# revision 40
# speedup vs baseline: 1.0938x; 1.0563x over previous
"""Sliding-window GQA attention (soft-capped) on 8 TRN2 NeuronCores.

Problem: B=2, S=2048, H=32 q-heads, H_KV=8 kv-heads, D=128, causal sliding
window 1024, logits soft-cap 30*tanh(s/30), scale 1/sqrt(D).

Sharding: head-parallel. Core c gets kv head c and q heads [4c, 4c+4) —
fully independent per core, no collectives.

Per-core algorithm (all on one NeuronCore, Tile-scheduled):
  - Q^T/K^T layouts ([d, s]) built on-chip via PE transposes, cast to bf16.
  - Scores computed TRANSPOSED: for each k-tile kt, one strip
    S^T[k=128, q window <=1152] = K_tile^T.T @ Q^T — avoids transposing
    probabilities for the PV matmul.
  - Soft-cap+softmax without max-subtraction (logits bounded by +-30):
    t = tanh(s * scale/30) on ScalarE (PSUM->SBUF), E = exp(30 t) on
    ScalarE (-> bf16). Causal/window masks: multiply 2 boundary 128-col
    blocks by 0/1 masks on VectorE.
  - For each 512-wide q-chunk: num^T[d, q] = sum_kt V_kt.T.T @ E_strip
    accumulated in PSUM (per-element has_written handles the staggered
    strip windows); den[q] replicated across partitions via an all-ones
    stationary matmul. out = (num/den) transposed back via PE.
"""

import numpy as np

import concourse.bass as bass
import concourse.mybir as mybir
import concourse.tile as tile
from concourse import bacc
from concourse import bass_utils as _bu
from concourse.bass_utils import run_bass_kernel_spmd
from concourse.masks import make_identity



AF = mybir.ActivationFunctionType
F32 = mybir.dt.float32
BF16 = mybir.dt.bfloat16

P = 128  # head dim == partition count == seq tile
B = 2
S = 2048
QH = 4  # q heads per core
NT = S // P  # 16 seq tiles
W = 1024  # sliding window
MAXW = W + P  # max strip width (9 tiles)
CHUNK = 512
NCH = S // CHUNK  # q-chunks per (b, head)
SCALE = 1.0 / np.sqrt(128.0)
CAP = 30.0
N_CORES = 8


def _strip_width(kt: int) -> int:
    return min(MAXW, S - kt * P)


def build_core_graph():
    nc = bacc.Bacc("TRN2", target_bir_lowering=False, debug=False, num_devices=N_CORES)
    q_ext = nc.declare_dram_parameter("query", [B, S, QH * P], F32, isOutput=False)
    k_ext = nc.declare_dram_parameter("key", [B, S, P], F32, isOutput=False)
    v_ext = nc.declare_dram_parameter("value", [B, S, P], F32, isOutput=False)
    # out stays in the on-chip [d, q] orientation; the host permutes to
    # [B, S, QH*P] while unsharding.
    out_ext = nc.declare_dram_parameter("out", [B, QH, P, S], F32, isOutput=True)

    with tile.TileContext(nc) as tc:
        with (
            tc.tile_pool(name="const", bufs=1) as constp,
            tc.tile_pool(name="persist", bufs=1) as pp,
        ):
            ident = constp.tile([P, P], F32, name="ident", tag="ident")
            make_identity(nc, ident[:])
            ident_bf = constp.tile([P, P], BF16, name="identbf", tag="identbf")
            make_identity(nc, ident_bf[:])
            # den only needs one row (gpsimd broadcasts it); a 1-col
            # stationary makes its LDWEIGHTS nearly free.
            ones_bf = constp.tile([P, 1], BF16, name="ones", tag="ones")
            nc.vector.memset(ones_bf[:], 1.0)
            # Strip coords: row r = k offset, col c = q offset (q-k = c-r).
            # m1 (first 128 cols): keep c >= r (causal).
            m1 = constp.tile([P, P], BF16, name="m1", tag="m1")
            nc.gpsimd.memset(m1[:], 1.0)
            nc.gpsimd.affine_select(
                out=m1[:],
                in_=m1[:],
                compare_op=mybir.AluOpType.is_ge,
                fill=0.0,
                base=0,
                pattern=[[1, P]],
                channel_multiplier=-1,
            )
            # m2 (cols [1024,1152)): keep c' < r (window cutoff at c-r=1024).
            m2 = constp.tile([P, P], BF16, name="m2", tag="m2")
            nc.gpsimd.memset(m2[:], 1.0)
            nc.gpsimd.affine_select(
                out=m2[:],
                in_=m2[:],
                compare_op=mybir.AluOpType.is_gt,
                fill=0.0,
                base=0,
                pattern=[[-1, P]],
                channel_multiplier=1,
            )

            # dummy transcendentals up front so the ~2.7us ACT table load
            # lands in the startup shadow, not before the first real tanh
            warm = constp.tile([P, 1], F32, name="warm", tag="warm")
            nc.scalar.activation(warm[:], ones_bf[:, 0:1], AF.Tanh)
            nc.scalar.activation(warm[:], warm[:], AF.Exp)

            # Persistent bf16 layouts. qT_all[b] holds the 4 heads
            # concatenated: head h occupies cols [h*S, (h+1)*S).
            qT_all = [
                pp.tile([P, QH * S], BF16, name=f"qT{b}", tag=f"qT{b}") for b in range(B)
            ]
            qT = [
                [qT_all[b][:, h * S : (h + 1) * S] for h in range(QH)] for b in range(B)
            ]
            kT = [pp.tile([P, S], BF16, name=f"kT{b}", tag=f"kT{b}") for b in range(B)]
            vB = [pp.tile([P, S], BF16, name=f"vB{b}", tag=f"vB{b}") for b in range(B)]

            # ---- main loop (loads/transposes folded in on demand so the
            # first strip starts as soon as ~9 Q tiles have landed) ----
            with (
                tc.tile_pool(name="load", bufs=6) as loadp,
                tc.tile_pool(name="spsum", bufs=2, space="PSUM") as sp,
                tc.tile_pool(name="apsum", bufs=2, space="PSUM") as auxp,
                tc.tile_pool(name="tbuf", bufs=2) as tbp,
                tc.tile_pool(name="ebuf", bufs=13) as ebp,
                tc.tile_pool(name="misc", bufs=2) as mp,
            ):
                loaded = set()
                loaded_k = set()

                def ensure_k(b, t):
                    if (b, t) in loaded_k or t >= NT:
                        return
                    loaded_k.add((b, t))
                    rows = slice(t * P, (t + 1) * P)
                    ktile = loadp.tile([P, P], F32, name="kload", tag="kload")
                    nc.sync.dma_start(out=ktile[:], in_=k_ext[b, rows, :])
                    kcast = loadp.tile([P, P], BF16, name="kcast", tag="kcast")
                    nc.vector.tensor_copy(kcast[:], ktile[:])
                    kps = auxp.tile([P, P], BF16, name="kps", tag="aux")
                    nc.tensor.transpose(kps[:], kcast[:], ident_bf[:])
                    nc.vector.tensor_copy(kT[b][:, t * P : (t + 1) * P], kps[:])

                def fast_start(b, nt9=9):
                    # startup: head-0 Q and K for tiles [0, nt9) in ONE
                    # strided DMA each, then xbar-transpose tile-by-tile.
                    q_re = q_ext[b].rearrange("(t p) d -> p t d", p=P)[:, 0:nt9, 0:P]
                    k_re = k_ext[b].rearrange("(t p) d -> p t d", p=P)[:, 0:nt9, :]
                    for name, src, dst_all in (
                        ("qf", q_re, qT_all[b]),
                        ("kf", k_re, kT[b]),
                    ):
                        raw = loadp.tile([P, nt9 * P], F32, name=f"{name}raw", tag="qload")
                        nc.sync.dma_start(
                            out=raw.rearrange("p (t d) -> p t d", d=P), in_=src
                        )
                        cast = loadp.tile([P, nt9 * P], BF16, name=f"{name}c", tag="qcast")
                        nc.vector.tensor_copy(cast[:], raw[:])
                        for g in range(0, nt9, 4):
                            gn = min(4, nt9 - g)
                            ps = auxp.tile([P, 4 * P], BF16, name=f"{name}ps", tag="aux")
                            for j in range(gn):
                                t = g + j
                                nc.tensor.transpose(
                                    ps[:, j * P : (j + 1) * P],
                                    cast[:, t * P : (t + 1) * P],
                                    ident_bf[:],
                                )
                            nc.vector.tensor_copy(
                                dst_all[:, g * P : (g + gn) * P], ps[:, : gn * P]
                            )
                    for t in range(nt9):
                        loaded_k.add((b, t))

                def ensure_tile(b, t, h0=0):
                    if (b, t) in loaded or t >= NT:
                        return
                    loaded.add((b, t))
                    nh = QH - h0
                    rows = slice(t * P, (t + 1) * P)
                    qtile = loadp.tile([P, nh * P], F32, name="qload", tag="qload")
                    nc.sync.dma_start(out=qtile[:], in_=q_ext[b, rows, h0 * P :])
                    qcast = loadp.tile([P, nh * P], BF16, name="qcast", tag="qcast")
                    nc.vector.tensor_copy(qcast[:], qtile[:])
                    qps = auxp.tile([P, nh * P], BF16, name="qps", tag="aux")
                    for i in range(nh):
                        nc.tensor.transpose(
                            qps[:, i * P : (i + 1) * P],
                            qcast[:, i * P : (i + 1) * P],
                            ident_bf[:],
                        )
                    # one strided copy scatters the head blocks
                    dst = qT_all[b].rearrange("p (h s) -> p h s", h=QH)[
                        :, h0:, t * P : (t + 1) * P
                    ]
                    nc.vector.tensor_copy(
                        dst, qps.rearrange("p (h s) -> p h s", h=nh)
                    )
                    ensure_k(b, t)
                    vtile = loadp.tile([P, P], F32, name="vload", tag="vload")
                    nc.sync.dma_start(out=vtile[:], in_=v_ext[b, rows, :])
                    nc.vector.tensor_copy(vB[b][:, t * P : (t + 1) * P], vtile[:])

                estrips = {}  # (b,h,kt) -> (epair_tile, col offset)
                tpairs = {}  # (b,h,pair) -> (tpair_tile, epair_tile)

                def emit_strip(b, h, kt):
                    w = _strip_width(kt)
                    q0s = kt * P  # strip q origin
                    strip = sp.tile([P, MAXW], F32, name="strip", tag="strip")
                    for c0 in range(0, w, CHUNK):
                        c1 = min(c0 + CHUNK, w)
                        nc.tensor.matmul(
                            strip[:, c0:c1],
                            lhsT=kT[b][:, q0s : q0s + P],
                            rhs=qT[b][h][:, q0s + c0 : q0s + c1],
                            start=True,
                            stop=True,
                        )
                    if kt % 2 == 0:
                        tpair = tbp.tile([P, 2 * MAXW], F32, name="t", tag="t")
                        epair = ebp.tile([P, 2 * MAXW], BF16, name="e", tag="e")
                        tpairs[(b, h, kt // 2)] = (tpair, epair)
                        off = 0
                    else:
                        tpair, epair = tpairs[(b, h, kt // 2)]
                        off = _strip_width(kt - 1)
                    estrips[(b, h, kt)] = (epair, off)
                    nc.scalar.activation(
                        tpair[:, off : off + w], strip[:, :w], AF.Tanh, scale=SCALE / CAP
                    )
                    if kt % 2 == 1:
                        # one merged exp for the strip pair, then masks
                        wtot = off + w
                        nc.scalar.activation(
                            epair[:, :wtot], tpair[:, :wtot], AF.Exp, scale=CAP
                        )
                        for k3 in (kt - 1, kt):
                            _, o3 = estrips[(b, h, k3)]
                            w3 = _strip_width(k3)
                            nc.vector.tensor_mul(
                                epair[:, o3 : o3 + P], epair[:, o3 : o3 + P], m1[:]
                            )
                            if w3 > W:
                                nc.vector.tensor_mul(
                                    epair[:, o3 + W : o3 + W + P],
                                    epair[:, o3 + W : o3 + W + P],
                                    m2[:],
                                )

                def _chunk_mms(b, h, c, dst, lhs_of, half):
                    q0 = c * CHUNK
                    kts = list(range(max(0, 4 * c - 8), 4 * c + 4))
                    mid = (len(kts) + 1) // 2
                    sel = kts[:mid] if half == 0 else kts[mid:]
                    for k2 in sel:
                        s0 = max(q0, k2 * P)
                        s1 = min(q0 + CHUNK, k2 * P + _strip_width(k2))
                        col0 = s0 - k2 * P
                        n = s1 - s0
                        d0 = s0 - q0
                        lhs = lhs_of(k2)
                        etile, eoff = estrips[(b, h, k2)]
                        nc.tensor.matmul(
                            dst[: lhs.shape[1], d0 : d0 + n],
                            lhsT=lhs,
                            rhs=etile[:, eoff + col0 : eoff + col0 + n],
                            start=(k2 == kts[0]),
                            stop=(k2 == kts[-1]),
                            skip_group_check=True,
                        )

                def chunk_recip(st):
                    # reciprocal of the single den row, gpsimd-broadcast
                    # across partitions; runs while the num matmuls stream
                    recip_row = mp.tile([1, CHUNK], F32, name="recip_row", tag="recip_row")
                    nc.vector.reciprocal_approx_fast(recip_row[:], st["den"][:])
                    recip_rep = mp.tile([P, CHUNK], F32, name="recip_rep", tag="recip_rep")
                    nc.gpsimd.partition_broadcast(recip_rep[:], recip_row[:])
                    return recip_rep

                def chunk_tail(st):
                    b, h, c = st["key"]
                    q0 = c * CHUNK
                    # one TT multiply draining num PSUM -> SBUF, one DMA out
                    ostage = mp.tile([P, CHUNK], F32, name="ostage", tag="ostage")
                    nc.vector.tensor_mul(ostage[:], st["num"][:], st["recip_rep"][:])
                    nc.sync.dma_start(
                        out=out_ext[b, h, :, q0 : q0 + CHUNK],
                        in_=ostage[:],
                    )

                pending = []

                def advance_pending():
                    # den first: its recip/broadcast tail overlaps the num
                    # matmuls, so the final TT fires right after num lands
                    if not pending:
                        return
                    st = pending[0]
                    b, h, c = st["key"]
                    stage = st["stage"]
                    if stage == 0:
                        st["den"] = auxp.tile([1, CHUNK], F32, name="den", tag="aux")
                        _chunk_mms(b, h, c, st["den"], lambda k2: ones_bf[:], 0)
                    elif stage == 1:
                        _chunk_mms(b, h, c, st["den"], lambda k2: ones_bf[:], 1)
                        st["recip_rep"] = chunk_recip(st)
                    elif stage == 2:
                        st["num"] = auxp.tile([P, CHUNK], F32, name="num", tag="aux")
                        _chunk_mms(b, h, c, st["num"],
                                   lambda k2: vB[b][:, k2 * P : (k2 + 1) * P], 0)
                    else:
                        _chunk_mms(b, h, c, st["num"],
                                   lambda k2: vB[b][:, k2 * P : (k2 + 1) * P], 1)
                        chunk_tail(st)
                        pending.pop(0)
                        return
                    st["stage"] = stage + 1

                for b in range(B):
                    for h in range(QH):
                        for kt in range(NT):
                            if b == 0 and h == 0:
                                if kt == 0:
                                    fast_start(0)
                                else:
                                    ensure_tile(0, kt - 1, h0=1)
                                    ensure_tile(0, kt + 8)
                            elif h == 0 and kt > 0:
                                ensure_tile(b, kt + 8)
                            emit_strip(b, h, kt)
                            # chunk work trickles in between strips so PE
                            # never runs a long chunk block right before a
                            # strip ACT depends on
                            advance_pending()
                            if kt % 4 == 3:
                                pending.append({"key": (b, h, kt // 4), "stage": 0})
                            if h == QH - 2 and b + 1 < B:
                                ensure_tile(b + 1, 2 * kt)
                                ensure_tile(b + 1, 2 * kt + 1)
                while pending:
                    advance_pending()
    nc.compile()
    return nc


_NC_CACHE = [None]


def _get_nc():
    if _NC_CACHE[0] is None:
        _NC_CACHE[0] = build_core_graph()
    return _NC_CACHE[0]


def _shard(query, key, value):
    in_maps = []
    for c in range(N_CORES):
        in_maps.append(
            {
                "query": np.ascontiguousarray(
                    query[:, :, c * QH * P : (c + 1) * QH * P], dtype=np.float32
                ),
                "key": np.ascontiguousarray(
                    key[:, :, c * P : (c + 1) * P], dtype=np.float32
                ),
                "value": np.ascontiguousarray(
                    value[:, :, c * P : (c + 1) * P], dtype=np.float32
                ),
            }
        )
    return in_maps


def _run(query, key, value, trace=False):
    nc = _get_nc()
    in_maps = _shard(query, key, value)
    res = run_bass_kernel_spmd(nc, in_maps, core_ids=list(range(N_CORES)), trace=trace)
    out = np.empty((B, S, N_CORES * QH * P), dtype=np.float32)
    for c in range(N_CORES):
        # device output is [B, QH, P(d), S]; permute to [B, S, QH*P]
        o = res.results[c]["out"].transpose(0, 3, 1, 2).reshape(B, S, QH * P)
        out[:, :, c * QH * P : (c + 1) * QH * P] = o
    return out, res


def kernel(query, key, value):
    out, _ = _run(query, key, value, trace=False)
    return out
